# revision 7
# baseline (speedup 1.0000x reference)
"""TRN2 Bass kernel for nn_AttributeClassifierHeaders (dense per-head MLP).

Computes y[b, a] = sigmoid(gelu(x @ W1[a] + b1[a]) . W2[a] + b2[a]) for 40
heads, sharded 5 heads per NeuronCore across 8 cores (head-parallel: each
head's weights are independent; x is replicated).

Stage 1 runs on the PE in fp8e4 with perf_mode=DoubleRow (2 fp8 weights per
cell, K=256 per matmul instruction, 2x the bf16/f32r FLOP rate; measured
216 ns net per [128, 512] matmul = ~96% of the 157 TF/s fp8 peak).
Host-side, x is scaled by 2^4 and W1 by 2^13 so every operand magnitude
stays below the TRN e4m3 max-normal of 240; the product's 2^17 scale is
divided back out by the gelu activation's `scale` argument. fp8
quantization puts the end-to-end max error at ~1.3e-2 of the sigmoid range
(gate: 2e-2); stage 2 therefore runs in bf16 (h is written as bf16 directly
by the gelu ACT, W2 kept bf16) so it adds no meaningful extra error, and
its sigmoid runs on ScalarE straight out of PSUM.

All inputs are SBUF-resident (x8 64 KiB/partition, W1 80 KiB/partition).
The batch is processed in four 1024-column quarters; in the repeat
(timing) build every input byte is still re-DMA'd once per iteration, but
each reload is emitted right after its last reader inside the iteration
(x quarter q after section q, W1 head a after its section-3 block) so
reloads overlap compute instead of stalling the PE behind the For_i
all-engine barrier -- the reloads whose last reader is the end of the
iteration (x quarter 3, W1 head 4, biases) run at the top of the next
iteration instead, where their consumers are a quarter/section away. The
first iteration's full input load is the same DMA set gated on iv == 0
(a skipped conditional DMA still bumps its semaphore, keeping the
scheduler's counts identical every iteration). Stage-2 matmuls are
emitted one (quarter, head) group late so the in-order PE queue never
waits on the gelu that produces their rhs.
"""
import os
import sys
from contextlib import ExitStack

import numpy as np
import ml_dtypes

for _p in ("/root/.axon_site/_ro/trn_rl_repo", "/opt/trn_rl_repo"):
    if os.path.isdir(_p) and _p not in sys.path:
        sys.path.append(_p)

import jax  # noqa: E402
from jax.sharding import Mesh, PartitionSpec, NamedSharding  # noqa: E402
from jax.experimental.shard_map import shard_map  # noqa: E402

import concourse.bacc as bacc  # noqa: E402
import concourse.tile as tile  # noqa: E402
from concourse import mybir, bass2jax  # noqa: E402

F32 = mybir.dt.float32
BF16 = mybir.dt.bfloat16
FP8 = mybir.dt.float8e4
AF = mybir.ActivationFunctionType
DR = mybir.MatmulPerfMode.DoubleRow

# problem shape (hardcoded; see module docstring)
B, D, A, H = 4096, 2048, 40, 1024
NCORES = 8
APC = A // NCORES        # 5 heads per core
KS = D // 256            # 8 double-slabs (K=256 per DoubleRow matmul)
MB = H // 128            # 8 hid blocks of 128
NQ = 4                   # 1024-col batch quarters (outer loop sections)
NBJ = B // NQ // 512     # 512-wide chunks per quarter (2)
QW = B // NQ             # quarter width (1024)

SX = 16.0                # x fp8 scale   (max |x*16|    ~ 82  < 240)
SW1 = 8192.0             # W1 fp8 scale  (max |W1*8192| ~ 181 < 240)
DQ1 = 1.0 / (SX * SW1)   # stage-1 dequant, folded into gelu's input scale


def build_program(repeat: int = 0):
    nc = bacc.Bacc("TRN2", target_bir_lowering=False, debug=False)
    x8_d = nc.dram_tensor("x8p", [KS, 128, 2, B], FP8,
                          kind="ExternalInput").ap()
    w1_d = nc.dram_tensor("w1p8", [APC, MB, 128, KS * 2 * 128], FP8,
                          kind="ExternalInput").ap()
    b1_d = nc.dram_tensor("b1p", [APC, 128, MB], F32, kind="ExternalInput").ap()
    w2_d = nc.dram_tensor("w2p", [APC, 128, MB], BF16, kind="ExternalInput").ap()
    b2_d = nc.dram_tensor("b2p", [1, APC], F32, kind="ExternalInput").ap()
    y_d = nc.dram_tensor("y", [APC, B], F32, kind="ExternalOutput").ap()

    with tile.TileContext(nc) as tc, ExitStack() as ctx:
        const = ctx.enter_context(tc.tile_pool(name="const", bufs=1))
        xp = ctx.enter_context(tc.tile_pool(name="xp", bufs=1))
        wp = ctx.enter_context(tc.tile_pool(name="wp", bufs=1))
        hp = ctx.enter_context(tc.tile_pool(name="hp", bufs=2))
        ysp = ctx.enter_context(tc.tile_pool(name="ysp", bufs=4))
        ps1 = ctx.enter_context(tc.tile_pool(name="ps1", bufs=4, space="PSUM"))
        ps2 = ctx.enter_context(tc.tile_pool(name="ps2", bufs=2, space="PSUM"))

        def body(cond_first=None, tails=False):
            """One full evaluation.

            cond_first: ScalarValue gating the full input load (None =
            unconditional). tails: emit per-section input reloads for the
            next iteration (repeat builds only).
            """
            b1t = const.tile([128, APC * MB], F32, tag="b1t", name="b1t")
            w2t = const.tile([128, APC * MB], BF16, tag="w2t", name="w2t")
            b2t = const.tile([1, APC], F32, tag="b2t", name="b2t")
            x8 = [xp.tile([128, 2, B], FP8, tag=f"x8_{s}", name=f"x8_{s}")
                  for s in range(KS)]
            w18 = [[wp.tile([128, KS, 2, 128], FP8, tag=f"w{a}_{m}",
                            name=f"w{a}_{m}") for m in range(MB)]
                   for a in range(APC)]

            def load_consts():
                for a in range(APC):
                    nc.sync.dma_start(b1t[:, a * MB:(a + 1) * MB], b1_d[a])
                    nc.sync.dma_start(w2t[:, a * MB:(a + 1) * MB], w2_d[a])
                nc.sync.dma_start(b2t[:], b2_d[:])

            def load_x_quarter(q):
                for s in range(KS):
                    nc.sync.dma_start(x8[s][:, :, q * QW:(q + 1) * QW],
                                      x8_d[s][:, :, q * QW:(q + 1) * QW])

            def load_w_head(a):
                for m in range(MB):
                    nc.sync.dma_start(w18[a][m][:], w1_d[a, m])

            # Full input load, first iteration only (sems still bump when
            # skipped, so every iteration has identical semaphore counts).
            for s in range(KS):
                nc.sync.dma_start(x8[s][:], x8_d[s], cond=cond_first)
            for a in range(APC):
                for m in range(MB):
                    nc.sync.dma_start(w18[a][m][:], w1_d[a, m],
                                      cond=cond_first)
            for a in range(APC):
                nc.sync.dma_start(b1t[:, a * MB:(a + 1) * MB], b1_d[a],
                                  cond=cond_first)
                nc.sync.dma_start(w2t[:, a * MB:(a + 1) * MB], w2_d[a],
                                  cond=cond_first)
            nc.sync.dma_start(b2t[:], b2_d[:], cond=cond_first)
            if tails:
                # reloads whose last reader was the END of the previous
                # iteration: consumers are a section away, so they overlap
                # this iteration's early compute.
                load_x_quarter(NQ - 1)
                load_w_head(APC - 1)
                load_consts()

            pending = []

            def stage2(q, a, ht):
                for j in range(NBJ):
                    psy = ps2.tile([1, 512], F32, tag="psy", name="psy")
                    for m in range(MB):
                        nc.tensor.matmul(
                            psy[:],
                            w2t[:, a * MB + m:a * MB + m + 1],
                            ht[m][:, j * 512:(j + 1) * 512],
                            start=(m == 0), stop=(m == MB - 1))
                    ys = ysp.tile([1, 512], F32, tag="ys", name="ys")
                    nc.scalar.activation(ys[:], psy[:], AF.Sigmoid,
                                         bias=b2t[:, a:a + 1])
                    off = q * QW + j * 512
                    nc.sync.dma_start(y_d[a:a + 1, off:off + 512], ys[:])

            for q in range(NQ):
                for a in range(APC):
                    ht = [hp.tile([128, QW], BF16, tag=f"h{m}", name=f"h{m}")
                          for m in range(MB)]
                    for m in range(MB):
                        if pending:
                            pending.pop(0)()
                        pt = [ps1.tile([128, 512], F32, tag="ps1",
                                       name=f"pt{j}") for j in range(NBJ)]
                        for s in range(KS):
                            for j in range(NBJ):
                                nb = q * QW + j * 512
                                nc.tensor.matmul(
                                    pt[j][:], w18[a][m][:, s],
                                    x8[s][:, :, nb:nb + 512],
                                    start=(s == 0), stop=(s == KS - 1),
                                    perf_mode=DR)
                        for j in range(NBJ):
                            nc.scalar.activation(
                                ht[m][:, j * 512:(j + 1) * 512],
                                pt[j][:], AF.Gelu,
                                bias=b1t[:, a * MB + m:a * MB + m + 1],
                                scale=DQ1)
                    pending.append(lambda q=q, a=a, ht=ht: stage2(q, a, ht))
                    if tails and q == NQ - 1 and a < APC - 1:
                        load_w_head(a)
                if tails and q < NQ - 1:
                    load_x_quarter(q)
            while pending:
                pending.pop(0)()

        if repeat and repeat > 1:
            with tc.For_i(0, repeat, 1) as iv:
                body(cond_first=(iv < 1), tails=True)
        else:
            body()
    nc.compile()
    return nc


class _Runner:
    """jit-once PJRT runner for a prebuilt Bass program (8-core SPMD)."""

    def __init__(self, nc, n_cores):
        bass2jax.install_neuronx_cc_hook()
        self.nc = nc
        self.n_cores = n_cores
        in_names, out_names, out_avals, zero_outs = [], [], [], []
        for alloc in nc.m.functions[0].allocations:
            if not isinstance(alloc, mybir.MemoryLocationSet):
                continue
            name = alloc.memorylocations[0].name
            if alloc.kind == "ExternalInput":
                in_names.append(name)
            elif alloc.kind == "ExternalOutput":
                shape = tuple(alloc.tensor_shape)
                dtype = mybir.dt.np(alloc.dtype)
                out_names.append(name)
                out_avals.append(jax.core.ShapedArray(shape, dtype))
                zero_outs.append(np.zeros(shape, dtype))
        partition_name = (nc.partition_id_tensor.name
                          if nc.partition_id_tensor else None)
        if partition_name is not None and partition_name in in_names:
            in_names.remove(partition_name)
        self.in_names = in_names
        self.out_names = out_names
        self.zero_outs = zero_outs
        n_params = len(in_names)
        n_outs = len(out_avals)
        all_in_names = list(in_names) + list(out_names)
        if partition_name is not None:
            all_in_names.append(partition_name)
        donate = tuple(range(n_params, n_params + n_outs))

        def _body(*args):
            operands = list(args)
            if partition_name is not None:
                operands.append(bass2jax.partition_id_tensor())
            outs = bass2jax._bass_exec_p.bind(
                *operands,
                out_avals=tuple(out_avals),
                in_names=tuple(all_in_names),
                out_names=tuple(out_names),
                lowering_input_output_aliases=(),
                sim_require_finite=True,
                sim_require_nnan=True,
                nc=nc,
            )
            return tuple(outs)

        devices = jax.devices()[:n_cores]
        assert len(devices) == n_cores, f"need {n_cores} neuron cores"
        self.mesh = Mesh(np.asarray(devices), ("core",))
        in_specs = (PartitionSpec("core"),) * (n_params + n_outs)
        out_specs = (PartitionSpec("core"),) * n_outs
        self.fn = jax.jit(
            shard_map(_body, mesh=self.mesh, in_specs=in_specs,
                      out_specs=out_specs, check_rep=False),
            donate_argnums=donate, keep_unused=True,
        )
        self._dev_inputs = None

    def put_inputs(self, in_maps):
        sharding = NamedSharding(self.mesh, PartitionSpec("core"))
        self._dev_inputs = [
            jax.device_put(
                np.concatenate([np.asarray(m[name]) for m in in_maps], axis=0),
                sharding)
            for name in self.in_names
        ]

    def run(self):
        sharding = NamedSharding(self.mesh, PartitionSpec("core"))
        zouts = [jax.device_put(np.concatenate([z] * self.n_cores, axis=0),
                                sharding) for z in self.zero_outs]
        outs = self.fn(*self._dev_inputs, *zouts)
        jax.block_until_ready(outs)
        return outs

    def run_np(self):
        outs = self.run()
        res = []
        for c in range(self.n_cores):
            d = {}
            for i, name in enumerate(self.out_names):
                full = np.asarray(outs[i])
                per = full.shape[0] // self.n_cores
                d[name] = full[c * per:(c + 1) * per]
            res.append(d)
        return res


_CACHE = {}


def _get_runner(repeat=0):
    if repeat not in _CACHE:
        _CACHE[repeat] = _Runner(build_program(repeat), NCORES)
    return _CACHE[repeat]


E4NP = ml_dtypes.float8_e4m3fn
BF16NP = ml_dtypes.bfloat16


def make_in_maps(x, W1, b1, W2, b2):
    x = np.asarray(x, dtype=np.float32)
    W1 = np.asarray(W1, dtype=np.float32)
    b1 = np.asarray(b1, dtype=np.float32)
    W2 = np.asarray(W2, dtype=np.float32)
    b2 = np.asarray(b2, dtype=np.float32)
    # x8p[s, p, i, n] = e4m3(16 * x[n, 256s+128i+p])
    x8 = (x.T * SX).astype(E4NP)                       # [D, B]
    x8p = np.ascontiguousarray(
        x8.reshape(KS, 2, 128, B).transpose(0, 2, 1, 3))
    # w1p8[a, mb, p, s, i, m] = e4m3(8192 * W1[a, 256s+128i+p, 128mb+m])
    w18 = (W1 * SW1).astype(E4NP)                      # [A, D, H]
    w1p8 = np.ascontiguousarray(
        w18.reshape(A, KS, 2, 128, MB, 128).transpose(0, 4, 3, 1, 2, 5)
        .reshape(A, MB, 128, KS * 2 * 128))
    b1p = np.ascontiguousarray(b1.reshape(A, MB, 128).transpose(0, 2, 1))
    w2p = np.ascontiguousarray(
        W2.reshape(A, MB, 128).transpose(0, 2, 1)).astype(BF16NP)
    b2p = np.ascontiguousarray(b2.reshape(A // APC, 1, APC))
    in_maps = []
    for c in range(NCORES):
        s = slice(c * APC, (c + 1) * APC)
        in_maps.append({"x8p": x8p, "w1p8": w1p8[s], "b1p": b1p[s],
                        "w2p": w2p[s], "b2p": b2p[c]})
    return in_maps


def kernel(x, W1, b1, W2, b2):
    in_maps = make_in_maps(x, W1, b1, W2, b2)
    r = _get_runner(0)
    r.put_inputs(in_maps)
    outs = r.run_np()
    y = np.concatenate([outs[c]["y"] for c in range(NCORES)], axis=0)
    return np.ascontiguousarray(y.T).astype(np.float32)


# revision 8
# speedup vs baseline: 1.2460x; 1.2460x over previous
"""TRN2 Bass kernel for nn_AttributeClassifierHeaders (dense per-head MLP).

Computes y[b, a] = sigmoid(gelu(x @ W1[a] + b1[a]) . W2[a] + b2[a]) for 40
heads, sharded 5 heads per NeuronCore across 8 cores (head-parallel: each
head's weights are independent; x is replicated).

Stage 1 runs on the PE in fp8e4 with perf_mode=DoubleRow (2 fp8 weights per
cell, K=256 per matmul instruction, 2x the bf16/f32r FLOP rate; measured
216 ns net per [128, 512] matmul = ~96% of the 157 TF/s fp8 peak).
Host-side, x is scaled by 2^4 and W1 by 2^13 so every operand magnitude
stays below the TRN e4m3 max-normal of 240; the product's 2^17 scale is
divided back out by the gelu activation's `scale` argument. fp8
quantization puts the end-to-end max error at ~1.3e-2 of the sigmoid range
(gate: 2e-2); stage 2 therefore runs in bf16 (h is written as bf16 directly
by the gelu ACT, W2 kept bf16) so it adds no meaningful extra error, and
its sigmoid runs on ScalarE straight out of PSUM.

All inputs are SBUF-resident (x8 64 KiB/partition, W1 80 KiB/partition).
The batch is processed in four 1024-column quarters; in the repeat
(timing) build every input byte is still re-DMA'd once per iteration, but
each reload is emitted right after its last reader inside the iteration
(x quarter q after section q, W1 head a after its section-3 block) so
reloads overlap compute instead of stalling the PE behind the For_i
all-engine barrier -- the reloads whose last reader is the end of the
iteration (x quarter 3, W1 head 4, biases) run at the top of the next
iteration instead, where their consumers are a quarter/section away. The
first iteration's full input load is the same DMA set gated on iv == 0
(a skipped conditional DMA still bumps its semaphore, keeping the
scheduler's counts identical every iteration). Stage-2 matmuls are
emitted one (quarter, head) group late so the in-order PE queue never
waits on the gelu that produces their rhs.
"""
import os
import sys
from contextlib import ExitStack

import numpy as np
import ml_dtypes

for _p in ("/root/.axon_site/_ro/trn_rl_repo", "/opt/trn_rl_repo"):
    if os.path.isdir(_p) and _p not in sys.path:
        sys.path.append(_p)

import jax  # noqa: E402
from jax.sharding import Mesh, PartitionSpec, NamedSharding  # noqa: E402
from jax.experimental.shard_map import shard_map  # noqa: E402

import concourse.bacc as bacc  # noqa: E402
import concourse.tile as tile  # noqa: E402
from concourse import mybir, bass2jax  # noqa: E402

F32 = mybir.dt.float32
BF16 = mybir.dt.bfloat16
FP8 = mybir.dt.float8e4
AF = mybir.ActivationFunctionType
DR = mybir.MatmulPerfMode.DoubleRow

# problem shape (hardcoded; see module docstring)
B, D, A, H = 4096, 2048, 40, 1024
NCORES = 8
APC = A // NCORES        # 5 heads per core
KS = D // 256            # 8 double-slabs (K=256 per DoubleRow matmul)
MB = H // 128            # 8 hid blocks of 128
NQ = 4                   # 1024-col batch quarters (outer loop sections)
NBJ = B // NQ // 512     # 512-wide chunks per quarter (2)
QW = B // NQ             # quarter width (1024)

SX = 16.0                # x fp8 scale   (max |x*16|    ~ 82  < 240)
SW1 = 8192.0             # W1 fp8 scale  (max |W1*8192| ~ 181 < 240)
DQ1 = 1.0 / (SX * SW1)   # stage-1 dequant, folded into gelu's input scale


def build_program(repeat: int = 0):
    nc = bacc.Bacc("TRN2", target_bir_lowering=False, debug=False)
    x8_d = nc.dram_tensor("x8p", [KS, 128, 2, B], FP8,
                          kind="ExternalInput").ap()
    w1_d = nc.dram_tensor("w1p8", [APC, MB, 128, KS * 2 * 128], FP8,
                          kind="ExternalInput").ap()
    b1_d = nc.dram_tensor("b1p", [APC, 128, MB], F32, kind="ExternalInput").ap()
    w2_d = nc.dram_tensor("w2p", [APC, 128, MB], BF16, kind="ExternalInput").ap()
    b2_d = nc.dram_tensor("b2p", [1, APC], F32, kind="ExternalInput").ap()
    y_d = nc.dram_tensor("y", [APC, B], F32, kind="ExternalOutput").ap()

    with tile.TileContext(nc) as tc, ExitStack() as ctx:
        const = ctx.enter_context(tc.tile_pool(name="const", bufs=1))
        xp = ctx.enter_context(tc.tile_pool(name="xp", bufs=1))
        wp = ctx.enter_context(tc.tile_pool(name="wp", bufs=1))
        hp = ctx.enter_context(tc.tile_pool(name="hp", bufs=2))
        ysp = ctx.enter_context(tc.tile_pool(name="ysp", bufs=4))
        ps1 = ctx.enter_context(tc.tile_pool(name="ps1", bufs=4, space="PSUM"))
        ps2 = ctx.enter_context(tc.tile_pool(name="ps2", bufs=2, space="PSUM"))

        def body(cond_first=False, tails=False, compute=True):
            """One full evaluation.

            cond_first: emit the full input load at the top (prologue /
            single-shot). tails: emit per-section input reloads for the
            next iteration (repeat builds only); reloads whose last reader
            is the iteration end run at the top of the next iteration.
            """
            b1t = const.tile([128, APC * MB], F32, tag="b1t", name="b1t")
            w2t = const.tile([128, APC * MB], BF16, tag="w2t", name="w2t")
            b2t = const.tile([1, APC], F32, tag="b2t", name="b2t")
            x8 = [xp.tile([128, 2, B], FP8, tag=f"x8_{s}", name=f"x8_{s}")
                  for s in range(KS)]
            w18 = [[wp.tile([128, KS, 2, 128], FP8, tag=f"w{a}_{m}",
                            name=f"w{a}_{m}") for m in range(MB)]
                   for a in range(APC)]

            def load_consts():
                for a in range(APC):
                    nc.sync.dma_start(b1t[:, a * MB:(a + 1) * MB], b1_d[a])
                    nc.sync.dma_start(w2t[:, a * MB:(a + 1) * MB], w2_d[a])
                nc.sync.dma_start(b2t[:], b2_d[:])

            def load_x_quarter(q):
                for s in range(KS):
                    nc.sync.dma_start(x8[s][:, :, q * QW:(q + 1) * QW],
                                      x8_d[s][:, :, q * QW:(q + 1) * QW])

            def load_w_head(a):
                for m in range(MB):
                    nc.sync.dma_start(w18[a][m][:], w1_d[a, m])

            if cond_first:
                # full input load (prologue / single-shot)
                load_consts()
                for s in range(KS):
                    nc.sync.dma_start(x8[s][:], x8_d[s])
                for a in range(APC):
                    load_w_head(a)
            if tails:
                # reloads whose last reader was the END of the previous
                # iteration: consumers are a quarter/section away, so they
                # overlap this iteration's early compute. consts go first;
                # the first gelu of the iteration waits on b1t.
                load_consts()
                load_x_quarter(NQ - 1)
                load_w_head(APC - 1)

            if not compute:
                return

            pending = []

            def stage2(q, a, ht):
                for j in range(NBJ):
                    psy = ps2.tile([1, 512], F32, tag="psy", name="psy")
                    for m in range(MB):
                        nc.tensor.matmul(
                            psy[:],
                            w2t[:, a * MB + m:a * MB + m + 1],
                            ht[m][:, j * 512:(j + 1) * 512],
                            start=(m == 0), stop=(m == MB - 1))
                    ys = ysp.tile([1, 512], F32, tag="ys", name="ys")
                    nc.scalar.activation(ys[:], psy[:], AF.Sigmoid,
                                         bias=b2t[:, a:a + 1])
                    off = q * QW + j * 512
                    nc.sync.dma_start(y_d[a:a + 1, off:off + 512], ys[:])

            for q in range(NQ):
                for a in range(APC):
                    ht = [hp.tile([128, QW], BF16, tag=f"h{m}", name=f"h{m}")
                          for m in range(MB)]
                    for m in range(MB):
                        if pending:
                            pending.pop(0)()
                        pt = [ps1.tile([128, 512], F32, tag="ps1",
                                       name=f"pt{j}") for j in range(NBJ)]
                        for s in range(KS):
                            for j in range(NBJ):
                                nb = q * QW + j * 512
                                nc.tensor.matmul(
                                    pt[j][:], w18[a][m][:, s],
                                    x8[s][:, :, nb:nb + 512],
                                    start=(s == 0), stop=(s == KS - 1),
                                    perf_mode=DR)
                        for j in range(NBJ):
                            nc.scalar.activation(
                                ht[m][:, j * 512:(j + 1) * 512],
                                pt[j][:], AF.Gelu,
                                bias=b1t[:, a * MB + m:a * MB + m + 1],
                                scale=DQ1)
                    pending.append(lambda q=q, a=a, ht=ht: stage2(q, a, ht))
                    if tails and q == NQ - 1 and a < APC - 1:
                        load_w_head(a)
                if tails and q < NQ - 1:
                    load_x_quarter(q)
            while pending:
                pending.pop(0)()

        if repeat and repeat > 1:
            # prologue outside the loop: allocate tiles + full input load
            body(cond_first=True, tails=False, compute=False)
            with tc.For_i(0, repeat, 1):
                body(cond_first=False, tails=True)
        else:
            body(cond_first=True)
    nc.compile()
    return nc


class _Runner:
    """jit-once PJRT runner for a prebuilt Bass program (8-core SPMD)."""

    def __init__(self, nc, n_cores):
        bass2jax.install_neuronx_cc_hook()
        self.nc = nc
        self.n_cores = n_cores
        in_names, out_names, out_avals, zero_outs = [], [], [], []
        for alloc in nc.m.functions[0].allocations:
            if not isinstance(alloc, mybir.MemoryLocationSet):
                continue
            name = alloc.memorylocations[0].name
            if alloc.kind == "ExternalInput":
                in_names.append(name)
            elif alloc.kind == "ExternalOutput":
                shape = tuple(alloc.tensor_shape)
                dtype = mybir.dt.np(alloc.dtype)
                out_names.append(name)
                out_avals.append(jax.core.ShapedArray(shape, dtype))
                zero_outs.append(np.zeros(shape, dtype))
        partition_name = (nc.partition_id_tensor.name
                          if nc.partition_id_tensor else None)
        if partition_name is not None and partition_name in in_names:
            in_names.remove(partition_name)
        self.in_names = in_names
        self.out_names = out_names
        self.zero_outs = zero_outs
        n_params = len(in_names)
        n_outs = len(out_avals)
        all_in_names = list(in_names) + list(out_names)
        if partition_name is not None:
            all_in_names.append(partition_name)
        donate = tuple(range(n_params, n_params + n_outs))

        def _body(*args):
            operands = list(args)
            if partition_name is not None:
                operands.append(bass2jax.partition_id_tensor())
            outs = bass2jax._bass_exec_p.bind(
                *operands,
                out_avals=tuple(out_avals),
                in_names=tuple(all_in_names),
                out_names=tuple(out_names),
                lowering_input_output_aliases=(),
                sim_require_finite=True,
                sim_require_nnan=True,
                nc=nc,
            )
            return tuple(outs)

        devices = jax.devices()[:n_cores]
        assert len(devices) == n_cores, f"need {n_cores} neuron cores"
        self.mesh = Mesh(np.asarray(devices), ("core",))
        in_specs = (PartitionSpec("core"),) * (n_params + n_outs)
        out_specs = (PartitionSpec("core"),) * n_outs
        self.fn = jax.jit(
            shard_map(_body, mesh=self.mesh, in_specs=in_specs,
                      out_specs=out_specs, check_rep=False),
            donate_argnums=donate, keep_unused=True,
        )
        self._dev_inputs = None

    def put_inputs(self, in_maps):
        sharding = NamedSharding(self.mesh, PartitionSpec("core"))
        self._dev_inputs = [
            jax.device_put(
                np.concatenate([np.asarray(m[name]) for m in in_maps], axis=0),
                sharding)
            for name in self.in_names
        ]

    def run(self):
        sharding = NamedSharding(self.mesh, PartitionSpec("core"))
        zouts = [jax.device_put(np.concatenate([z] * self.n_cores, axis=0),
                                sharding) for z in self.zero_outs]
        outs = self.fn(*self._dev_inputs, *zouts)
        jax.block_until_ready(outs)
        return outs

    def run_np(self):
        outs = self.run()
        res = []
        for c in range(self.n_cores):
            d = {}
            for i, name in enumerate(self.out_names):
                full = np.asarray(outs[i])
                per = full.shape[0] // self.n_cores
                d[name] = full[c * per:(c + 1) * per]
            res.append(d)
        return res


_CACHE = {}


def _get_runner(repeat=0):
    if repeat not in _CACHE:
        _CACHE[repeat] = _Runner(build_program(repeat), NCORES)
    return _CACHE[repeat]


E4NP = ml_dtypes.float8_e4m3fn
BF16NP = ml_dtypes.bfloat16


def make_in_maps(x, W1, b1, W2, b2):
    x = np.asarray(x, dtype=np.float32)
    W1 = np.asarray(W1, dtype=np.float32)
    b1 = np.asarray(b1, dtype=np.float32)
    W2 = np.asarray(W2, dtype=np.float32)
    b2 = np.asarray(b2, dtype=np.float32)
    # x8p[s, p, i, n] = e4m3(16 * x[n, 256s+128i+p])
    x8 = (x.T * SX).astype(E4NP)                       # [D, B]
    x8p = np.ascontiguousarray(
        x8.reshape(KS, 2, 128, B).transpose(0, 2, 1, 3))
    # w1p8[a, mb, p, s, i, m] = e4m3(8192 * W1[a, 256s+128i+p, 128mb+m])
    w18 = (W1 * SW1).astype(E4NP)                      # [A, D, H]
    w1p8 = np.ascontiguousarray(
        w18.reshape(A, KS, 2, 128, MB, 128).transpose(0, 4, 3, 1, 2, 5)
        .reshape(A, MB, 128, KS * 2 * 128))
    b1p = np.ascontiguousarray(b1.reshape(A, MB, 128).transpose(0, 2, 1))
    w2p = np.ascontiguousarray(
        W2.reshape(A, MB, 128).transpose(0, 2, 1)).astype(BF16NP)
    b2p = np.ascontiguousarray(b2.reshape(A // APC, 1, APC))
    in_maps = []
    for c in range(NCORES):
        s = slice(c * APC, (c + 1) * APC)
        in_maps.append({"x8p": x8p, "w1p8": w1p8[s], "b1p": b1p[s],
                        "w2p": w2p[s], "b2p": b2p[c]})
    return in_maps


def kernel(x, W1, b1, W2, b2):
    in_maps = make_in_maps(x, W1, b1, W2, b2)
    r = _get_runner(0)
    r.put_inputs(in_maps)
    outs = r.run_np()
    y = np.concatenate([outs[c]["y"] for c in range(NCORES)], axis=0)
    return np.ascontiguousarray(y.T).astype(np.float32)


# revision 10
# speedup vs baseline: 1.3245x; 1.0631x over previous
"""TRN2 Bass kernel for nn_AttributeClassifierHeaders (dense per-head MLP).

Computes y[b, a] = sigmoid(gelu(x @ W1[a] + b1[a]) . W2[a] + b2[a]) for 40
heads, sharded 5 heads per NeuronCore across 8 cores (head-parallel: each
head's weights are independent; x is replicated).

Stage 1 runs on the PE in fp8e4 with perf_mode=DoubleRow (2 fp8 weights per
cell, K=256 per matmul instruction, 2x the bf16/f32r FLOP rate; measured
216 ns net per [128, 512] matmul = ~96% of the 157 TF/s fp8 peak).
Host-side, x is scaled by 2^4 and W1 by 2^13 so every operand magnitude
stays below the TRN e4m3 max-normal of 240; the product's 2^17 scale is
divided back out by the gelu activation's `scale` argument. fp8
quantization puts the end-to-end max error at ~1.3e-2 of the sigmoid range
(gate: 2e-2); stage 2 therefore runs in bf16 (h is written as bf16 directly
by the gelu ACT, W2 kept bf16) so it adds no meaningful extra error, and
its sigmoid runs on ScalarE straight out of PSUM.

All inputs are SBUF-resident (x8 64 KiB/partition, W1 80 KiB/partition).
The batch is processed in four 1024-column quarters; in the repeat
(timing) build every input byte is still re-DMA'd once per iteration, but
each reload is emitted right after its last reader inside the iteration
(x quarter q after section q, W1 head a after its section-3 block) so
reloads overlap compute instead of stalling the PE behind the For_i
all-engine barrier -- the reloads whose last reader is the end of the
iteration (x quarter 3, W1 head 4, biases) run at the top of the next
iteration instead, where their consumers are a quarter/section away. The
first iteration's full input load is the same DMA set gated on iv == 0
(a skipped conditional DMA still bumps its semaphore, keeping the
scheduler's counts identical every iteration). Stage-2 matmuls are
emitted one (quarter, head) group late so the in-order PE queue never
waits on the gelu that produces their rhs.
"""
import os
import sys
from contextlib import ExitStack

import numpy as np
import ml_dtypes

for _p in ("/root/.axon_site/_ro/trn_rl_repo", "/opt/trn_rl_repo"):
    if os.path.isdir(_p) and _p not in sys.path:
        sys.path.append(_p)

import jax  # noqa: E402
from jax.sharding import Mesh, PartitionSpec, NamedSharding  # noqa: E402
from jax.experimental.shard_map import shard_map  # noqa: E402

import concourse.bacc as bacc  # noqa: E402
import concourse.tile as tile  # noqa: E402
from concourse import mybir, bass2jax  # noqa: E402

F32 = mybir.dt.float32
BF16 = mybir.dt.bfloat16
FP8 = mybir.dt.float8e4
AF = mybir.ActivationFunctionType
DR = mybir.MatmulPerfMode.DoubleRow

# problem shape (hardcoded; see module docstring)
B, D, A, H = 4096, 2048, 40, 1024
NCORES = 8
APC = A // NCORES        # 5 heads per core
KS = D // 256            # 8 double-slabs (K=256 per DoubleRow matmul)
MB = H // 128            # 8 hid blocks of 128
NQ = 4                   # 1024-col batch quarters (outer loop sections)
NBJ = B // NQ // 512     # 512-wide chunks per quarter (2)
QW = B // NQ             # quarter width (1024)

SX = 16.0                # x fp8 scale   (max |x*16|    ~ 82  < 240)
SW1 = 8192.0             # W1 fp8 scale  (max |W1*8192| ~ 181 < 240)
DQ1 = 1.0 / (SX * SW1)   # stage-1 dequant, folded into gelu's input scale
SW2 = 4096.0             # W2 fp8 scale  (max |W2*4096| ~ 128 < 240)
DQ2 = 1.0 / SW2          # stage-2 dequant, folded into sigmoid's scale


def build_program(repeat: int = 0):
    nc = bacc.Bacc("TRN2", target_bir_lowering=False, debug=False)
    x8_d = nc.dram_tensor("x8p", [KS, 128, 2, B], FP8,
                          kind="ExternalInput").ap()
    w1_d = nc.dram_tensor("w1p8", [APC, MB, 128, KS * 2 * 128], FP8,
                          kind="ExternalInput").ap()
    b1_d = nc.dram_tensor("b1p", [APC, 128, MB], F32, kind="ExternalInput").ap()
    w2_d = nc.dram_tensor("w2p", [APC, MB // 2, 128, 2 * 16], FP8,
                          kind="ExternalInput").ap()
    b2_d = nc.dram_tensor("b2p", [1, APC], F32, kind="ExternalInput").ap()
    y_d = nc.dram_tensor("y", [APC, B], F32, kind="ExternalOutput").ap()

    with tile.TileContext(nc) as tc, ExitStack() as ctx:
        const = ctx.enter_context(tc.tile_pool(name="const", bufs=1))
        xp = ctx.enter_context(tc.tile_pool(name="xp", bufs=1))
        wp = ctx.enter_context(tc.tile_pool(name="wp", bufs=1))
        hp = ctx.enter_context(tc.tile_pool(name="hp", bufs=2))
        ysp = ctx.enter_context(tc.tile_pool(name="ysp", bufs=4))
        ps1 = ctx.enter_context(tc.tile_pool(name="ps1", bufs=4, space="PSUM"))
        ps2 = ctx.enter_context(tc.tile_pool(name="ps2", bufs=2, space="PSUM"))

        def body(cond_first=False, tails=False, compute=True):
            """One full evaluation.

            cond_first: emit the full input load at the top (prologue /
            single-shot). tails: emit per-section input reloads for the
            next iteration (repeat builds only); reloads whose last reader
            is the iteration end run at the top of the next iteration.
            """
            b1t = const.tile([128, APC * MB], F32, tag="b1t", name="b1t")
            w2t = [[const.tile([128, 2, 16], FP8, tag=f"w2_{a}_{s2}",
                               name=f"w2_{a}_{s2}") for s2 in range(MB // 2)]
                   for a in range(APC)]
            b2t = const.tile([1, APC], F32, tag="b2t", name="b2t")
            x8 = [xp.tile([128, 2, B], FP8, tag=f"x8_{s}", name=f"x8_{s}")
                  for s in range(KS)]
            w18 = [[wp.tile([128, KS, 2, 128], FP8, tag=f"w{a}_{m}",
                            name=f"w{a}_{m}") for m in range(MB)]
                   for a in range(APC)]

            def load_consts():
                for a in range(APC):
                    nc.sync.dma_start(b1t[:, a * MB:(a + 1) * MB], b1_d[a])
                    for s2 in range(MB // 2):
                        nc.sync.dma_start(w2t[a][s2][:], w2_d[a, s2])
                nc.sync.dma_start(b2t[:], b2_d[:])

            def load_x_quarter(q):
                for s in range(KS):
                    nc.sync.dma_start(x8[s][:, :, q * QW:(q + 1) * QW],
                                      x8_d[s][:, :, q * QW:(q + 1) * QW])

            def load_w_head(a):
                for m in range(MB):
                    nc.sync.dma_start(w18[a][m][:], w1_d[a, m])

            if cond_first:
                # full input load (prologue / single-shot)
                load_consts()
                for s in range(KS):
                    nc.sync.dma_start(x8[s][:], x8_d[s])
                for a in range(APC):
                    load_w_head(a)
            if tails:
                # reloads whose last reader was the END of the previous
                # iteration: consumers are a quarter/section away, so they
                # overlap this iteration's early compute. consts go first;
                # the first gelu of the iteration waits on b1t.
                load_consts()
                load_x_quarter(NQ - 1)
                load_w_head(APC - 1)

            if not compute:
                return

            pending = []

            def stage2(q, a, ht):
                for j in range(NBJ):
                    psy = ps2.tile([16, 512], F32, tag="psy", name="psy")
                    for s2 in range(MB // 2):
                        nc.tensor.matmul(
                            psy[:],
                            w2t[a][s2][:],
                            ht[s2][:, :, j * 512:(j + 1) * 512],
                            start=(s2 == 0), stop=(s2 == MB // 2 - 1),
                            perf_mode=DR)
                    ys = ysp.tile([1, 512], F32, tag="ys", name="ys")
                    nc.scalar.activation(ys[:], psy[0:1, :], AF.Sigmoid,
                                         bias=b2t[:, a:a + 1], scale=DQ2)
                    off = q * QW + j * 512
                    nc.sync.dma_start(y_d[a:a + 1, off:off + 512], ys[:])

            for q in range(NQ):
                for a in range(APC):
                    ht = [hp.tile([128, 2, QW], FP8, tag=f"h{s2}",
                                  name=f"h{s2}") for s2 in range(MB // 2)]
                    for m in range(MB):
                        if pending:
                            pending.pop(0)()
                        pt = [ps1.tile([128, 512], F32, tag="ps1",
                                       name=f"pt{j}") for j in range(NBJ)]
                        for s in range(KS):
                            for j in range(NBJ):
                                nb = q * QW + j * 512
                                nc.tensor.matmul(
                                    pt[j][:], w18[a][m][:, s],
                                    x8[s][:, :, nb:nb + 512],
                                    start=(s == 0), stop=(s == KS - 1),
                                    perf_mode=DR)
                        for j in range(NBJ):
                            nc.scalar.activation(
                                ht[m // 2][:, m % 2, j * 512:(j + 1) * 512],
                                pt[j][:], AF.Gelu,
                                bias=b1t[:, a * MB + m:a * MB + m + 1],
                                scale=DQ1)
                    pending.append(lambda q=q, a=a, ht=ht: stage2(q, a, ht))
                    if tails and q == NQ - 1 and a < APC - 1:
                        load_w_head(a)
                if tails and q < NQ - 1:
                    load_x_quarter(q)
            while pending:
                pending.pop(0)()

        if repeat and repeat > 1:
            # prologue outside the loop: allocate tiles + full input load
            body(cond_first=True, tails=False, compute=False)
            with tc.For_i(0, repeat, 1):
                body(cond_first=False, tails=True)
        else:
            body(cond_first=True)
    nc.compile()
    return nc


class _Runner:
    """jit-once PJRT runner for a prebuilt Bass program (8-core SPMD)."""

    def __init__(self, nc, n_cores):
        bass2jax.install_neuronx_cc_hook()
        self.nc = nc
        self.n_cores = n_cores
        in_names, out_names, out_avals, zero_outs = [], [], [], []
        for alloc in nc.m.functions[0].allocations:
            if not isinstance(alloc, mybir.MemoryLocationSet):
                continue
            name = alloc.memorylocations[0].name
            if alloc.kind == "ExternalInput":
                in_names.append(name)
            elif alloc.kind == "ExternalOutput":
                shape = tuple(alloc.tensor_shape)
                dtype = mybir.dt.np(alloc.dtype)
                out_names.append(name)
                out_avals.append(jax.core.ShapedArray(shape, dtype))
                zero_outs.append(np.zeros(shape, dtype))
        partition_name = (nc.partition_id_tensor.name
                          if nc.partition_id_tensor else None)
        if partition_name is not None and partition_name in in_names:
            in_names.remove(partition_name)
        self.in_names = in_names
        self.out_names = out_names
        self.zero_outs = zero_outs
        n_params = len(in_names)
        n_outs = len(out_avals)
        all_in_names = list(in_names) + list(out_names)
        if partition_name is not None:
            all_in_names.append(partition_name)
        donate = tuple(range(n_params, n_params + n_outs))

        def _body(*args):
            operands = list(args)
            if partition_name is not None:
                operands.append(bass2jax.partition_id_tensor())
            outs = bass2jax._bass_exec_p.bind(
                *operands,
                out_avals=tuple(out_avals),
                in_names=tuple(all_in_names),
                out_names=tuple(out_names),
                lowering_input_output_aliases=(),
                sim_require_finite=True,
                sim_require_nnan=True,
                nc=nc,
            )
            return tuple(outs)

        devices = jax.devices()[:n_cores]
        assert len(devices) == n_cores, f"need {n_cores} neuron cores"
        self.mesh = Mesh(np.asarray(devices), ("core",))
        in_specs = (PartitionSpec("core"),) * (n_params + n_outs)
        out_specs = (PartitionSpec("core"),) * n_outs
        self.fn = jax.jit(
            shard_map(_body, mesh=self.mesh, in_specs=in_specs,
                      out_specs=out_specs, check_rep=False),
            donate_argnums=donate, keep_unused=True,
        )
        self._dev_inputs = None

    def put_inputs(self, in_maps):
        sharding = NamedSharding(self.mesh, PartitionSpec("core"))
        self._dev_inputs = [
            jax.device_put(
                np.concatenate([np.asarray(m[name]) for m in in_maps], axis=0),
                sharding)
            for name in self.in_names
        ]

    def run(self):
        sharding = NamedSharding(self.mesh, PartitionSpec("core"))
        zouts = [jax.device_put(np.concatenate([z] * self.n_cores, axis=0),
                                sharding) for z in self.zero_outs]
        outs = self.fn(*self._dev_inputs, *zouts)
        jax.block_until_ready(outs)
        return outs

    def run_np(self):
        outs = self.run()
        res = []
        for c in range(self.n_cores):
            d = {}
            for i, name in enumerate(self.out_names):
                full = np.asarray(outs[i])
                per = full.shape[0] // self.n_cores
                d[name] = full[c * per:(c + 1) * per]
            res.append(d)
        return res


_CACHE = {}


def _get_runner(repeat=0):
    if repeat not in _CACHE:
        _CACHE[repeat] = _Runner(build_program(repeat), NCORES)
    return _CACHE[repeat]


E4NP = ml_dtypes.float8_e4m3fn
BF16NP = ml_dtypes.bfloat16


def make_in_maps(x, W1, b1, W2, b2):
    x = np.asarray(x, dtype=np.float32)
    W1 = np.asarray(W1, dtype=np.float32)
    b1 = np.asarray(b1, dtype=np.float32)
    W2 = np.asarray(W2, dtype=np.float32)
    b2 = np.asarray(b2, dtype=np.float32)
    # x8p[s, p, i, n] = e4m3(16 * x[n, 256s+128i+p])
    x8 = (x.T * SX).astype(E4NP)                       # [D, B]
    x8p = np.ascontiguousarray(
        x8.reshape(KS, 2, 128, B).transpose(0, 2, 1, 3))
    # w1p8[a, mb, p, s, i, m] = e4m3(8192 * W1[a, 256s+128i+p, 128mb+m])
    w18 = (W1 * SW1).astype(E4NP)                      # [A, D, H]
    w1p8 = np.ascontiguousarray(
        w18.reshape(A, KS, 2, 128, MB, 128).transpose(0, 4, 3, 1, 2, 5)
        .reshape(A, MB, 128, KS * 2 * 128))
    b1p = np.ascontiguousarray(b1.reshape(A, MB, 128).transpose(0, 2, 1))
    # w2p[a, s2, p, i, 0] = e4m3(4096 * W2[a, (2*s2+i)*128 + p]), padded to
    # 16 cols so the DoubleRow weights AP's pair-dim step is 16 bytes
    w28 = (W2 * SW2).astype(E4NP).reshape(A, MB // 2, 2, 128)
    w2p = np.zeros((A, MB // 2, 128, 2, 16), dtype=E4NP)
    w2p[..., 0] = w28.transpose(0, 1, 3, 2)
    w2p = np.ascontiguousarray(w2p.reshape(A, MB // 2, 128, 32))
    b2p = np.ascontiguousarray(b2.reshape(A // APC, 1, APC))
    in_maps = []
    for c in range(NCORES):
        s = slice(c * APC, (c + 1) * APC)
        in_maps.append({"x8p": x8p, "w1p8": w1p8[s], "b1p": b1p[s],
                        "w2p": w2p[s], "b2p": b2p[c]})
    return in_maps


def kernel(x, W1, b1, W2, b2):
    in_maps = make_in_maps(x, W1, b1, W2, b2)
    r = _get_runner(0)
    r.put_inputs(in_maps)
    outs = r.run_np()
    y = np.concatenate([outs[c]["y"] for c in range(NCORES)], axis=0)
    return np.ascontiguousarray(y.T).astype(np.float32)
